# revision 9
# baseline (speedup 1.0000x reference)
"""Trainium2 Bass kernel for nn_CrossAttention_Sp (moe_routing).

  W_k = U.T @ diag(S_k^2) @ U                       (host, tiny)
  x_k = x @ W_k ;  patch_k = softmax(x_k @ y.T * d^-0.5)
  q = softmax(coords @ pos_emb)  (shared over batch/k)
  attn_k = (1-g)*patch_k + g*q ;  e_k = -sum(attn_k*ln(attn_k+1e-8))
  route k* = argmin_k temp*e_k (ties k=0); heat = 2-2*sigmoid(min temp*e_k)
  out = attn_{k*} @ y

Data-parallel over B=16 across 8 cores. Matmuls in float32r (~4x faster
than fp32 on PE, ~1.6e-4 rel err). fp32r operand tiles are loaded via DMA
from fp32r DRAM params or written as fp32r by ACT/DVE (BIR verifier
requires producers of fp32r-matmul operands to emit fp32r).
"""

import numpy as np

import concourse.bass as bass
import concourse.mybir as mybir
import concourse.tile as tile
from concourse.bass_utils import run_bass_kernel_spmd

B, N, D = 16, 1024, 768
NCORES = 8
BPC = B // NCORES
NT = N // 128
DC = D // 128
F32 = mybir.dt.float32
F32R = mybir.dt.float32r
BF16 = mybir.dt.bfloat16
AF = mybir.ActivationFunctionType
OP = mybir.AluOpType


def _split_waits(nc, max_waits=1):
    """This toolchain's walrus accepts at most one sync-wait per instruction;
    split extras onto EventSemaphore nops on the same engine."""
    ctr = 0
    for func in nc.m.functions:
        for block in func.blocks:
            new_insts = []
            for inst in block.instructions:
                si = inst.sync_info
                if si is not None and len(si.on_wait) > max_waits:
                    waits = list(si.on_wait)
                    extra, keep = waits[:-max_waits], waits[-max_waits:]
                    while extra:
                        chunk, extra = extra[:max_waits], extra[max_waits:]
                        ctr += 1
                        new_insts.append(mybir.InstEventSemaphore(
                            name=f"I-waitsplit-{ctr}",
                            engine=inst.engine,
                            sync_info=mybir.SyncInfo(on_wait=chunk, on_update=[]),
                        ))
                    si.on_wait = keep
                new_insts.append(inst)
            block.instructions = new_insts
    return ctr


def build_nc(g: float, temp: float):
    one_m_g = 1.0 - g
    scale = float(D) ** -0.5

    nc = bass.Bass()
    xT_d = nc.declare_dram_parameter("xT", [BPC, D, N], F32R, isOutput=False)
    yT_d = nc.declare_dram_parameter("yT", [BPC, D, N], F32R, isOutput=False)
    y_d = nc.declare_dram_parameter("y", [BPC, N, D], F32R, isOutput=False)
    co_d = nc.declare_dram_parameter("coords", [N, N * 6], F32R, isOutput=False)
    dgs_d = nc.declare_dram_parameter("diags", [NT, 128, 6 * 128], F32R,
                                      isOutput=False)
    id_d = nc.declare_dram_parameter("ident", [128, 128], F32R, isOutput=False)
    w_d = nc.declare_dram_parameter("W", [2, D, D], F32R, isOutput=False)
    out_d = nc.declare_dram_parameter("out", [BPC, N, D], F32, isOutput=True)
    heat_d = nc.declare_dram_parameter("heat", [BPC, N], F32, isOutput=True)

    with tile.TileContext(nc) as tc:
        with (
            tc.tile_pool(name="singles", bufs=1) as singles,
            tc.tile_pool(name="gqp", bufs=1) as gqp,
            tc.tile_pool(name="psum_big", bufs=2, space="PSUM") as psum_big,
            tc.tile_pool(name="psum_out", bufs=2, space="PSUM") as psum_out,
        ):
            ident = singles.tile([128, 128], F32R, name="ident")
            nc.sync.dma_start(out=ident, in_=id_d[:])
            w_sb = singles.tile([128, 2, DC, D], F32R, name="w_sb")
            for k in range(2):
                nc.sync.dma_start(
                    out=w_sb[:, k],
                    in_=w_d[k].rearrange("(t p) d -> p t d", p=128),
                )
            heat_st = singles.tile([128, BPC * NT], F32, name="heat_st")
            eps_sb = singles.tile([128, 1], F32, name="eps_sb")
            nc.vector.memset(eps_sb, 1e-8)

            gq = [gqp.tile([128, N], BF16, name=f"gq{i}", tag=f"gq{i}")
                  for i in range(NT)]

            # ---- stage 1: pos scores (shared across batches and k) ----
            with (
                tc.tile_pool(name="s1", bufs=2) as s1,
                tc.tile_pool(name="s1s", bufs=2) as s1s,
            ):
                for nt in range(NT):
                    co_t = s1.tile([128, N * 6], F32R, name="co_t", tag="co")
                    nc.sync.dma_start(out=co_t,
                                      in_=co_d[nt * 128:(nt + 1) * 128, :])
                    co3 = co_t.rearrange("p (m c) -> p m c", c=6)
                    dg = s1s.tile([128, 6, 128], F32R, name="dg", tag="dg")
                    nc.sync.dma_start(out=dg, in_=dgs_d[nt].rearrange(
                        "p (c j) -> p c j", c=6))
                    ps = psum_big.tile([128, N], F32, name="ps_pos", tag="pbig")
                    for h in range(2):
                        for c in range(6):
                            nc.tensor.matmul(
                                ps[:, h * 512:(h + 1) * 512],
                                dg[:, c],
                                co3[:, h * 512:(h + 1) * 512, c],
                                start=(c == 0), stop=(c == 5),
                            )
                    qx = s1s.tile([128, N], F32, name="qx", tag="qx")
                    qden = s1s.tile([128, 1], F32, name="qden", tag="qden")
                    nc.scalar.activation(out=qx, in_=ps, func=AF.Exp,
                                         accum_out=qden)
                    qinv = s1s.tile([128, 1], F32, name="qinv", tag="qinv")
                    nc.vector.reciprocal(out=qinv, in_=qden)
                    nc.vector.tensor_scalar(
                        out=gq[nt], in0=qx, scalar1=qinv, scalar2=g,
                        op0=OP.mult, op1=OP.mult,
                    )

            # ---- stage 2: per batch ----
            with (
                tc.tile_pool(name="xy", bufs=2) as xy,
                tc.tile_pool(name="yp", bufs=1) as yp,
                tc.tile_pool(name="x12", bufs=1) as x12p,
                tc.tile_pool(name="work", bufs=1) as work,
            ):
                for b in range(BPC):
                    xt = xy.tile([128, DC, N], F32R, name="xt", tag="xyT")
                    nc.sync.dma_start(
                        out=xt, in_=xT_d[b].rearrange("(t p) m -> p t m", p=128))

                    x12 = [[x12p.tile([128, N], F32R, name=f"x{k}T{do}",
                                      tag=f"x{k}T{do}")
                            for do in range(DC)] for k in range(2)]
                    for k in range(2):
                        for do in range(DC):
                            psx = psum_big.tile([128, N], F32, name="psx",
                                                tag="pbig")
                            for di in range(DC):
                                for h in range(2):
                                    nc.tensor.matmul(
                                        psx[:, h * 512:(h + 1) * 512],
                                        w_sb[:, k, di, do * 128:(do + 1) * 128],
                                        xt[:, di, h * 512:(h + 1) * 512],
                                        start=(di == 0), stop=(di == DC - 1),
                                    )
                            nc.scalar.copy(out=x12[k][do], in_=psx[:])

                    yt = xy.tile([128, DC, N], F32R, name="yt", tag="xyT")
                    nc.sync.dma_start(
                        out=yt, in_=yT_d[b].rearrange("(t p) m -> p t m", p=128))
                    yn = yp.tile([128, NT, D], F32R, name="yn", tag="yn")
                    nc.sync.dma_start(
                        out=yn, in_=y_d[b].rearrange("(t p) d -> p t d", p=128))

                    for nt in range(NT):
                        attn = []
                        ents = []
                        for k in range(2):
                            psz = psum_big.tile([128, N], F32, name="psz",
                                                tag="pbig")
                            for dc in range(DC):
                                for h in range(2):
                                    nc.tensor.matmul(
                                        psz[:, h * 512:(h + 1) * 512],
                                        x12[k][dc][:, nt * 128:(nt + 1) * 128],
                                        yt[:, dc, h * 512:(h + 1) * 512],
                                        start=(dc == 0), stop=(dc == DC - 1),
                                    )
                            p_k = work.tile([128, N], F32, name=f"p{k}",
                                            tag=f"p{k}", bufs=2)
                            den = work.tile([128, 1], F32, name=f"den{k}",
                                            tag=f"den{k}", bufs=2)
                            nc.scalar.activation(out=p_k, in_=psz, func=AF.Exp,
                                                 scale=scale, accum_out=den)
                            inv = work.tile([128, 1], F32, name=f"inv{k}",
                                            tag=f"inv{k}", bufs=2)
                            nc.vector.reciprocal(out=inv, in_=den)
                            nc.vector.tensor_scalar(
                                out=p_k, in0=p_k, scalar1=inv, scalar2=one_m_g,
                                op0=OP.mult, op1=OP.mult,
                            )
                            nc.vector.tensor_add(out=p_k, in0=p_k, in1=gq[nt])
                            ln_t = work.tile([128, N], F32, name=f"ln{k}",
                                             tag="ln", bufs=1)
                            nc.scalar.activation(out=ln_t, in_=p_k, func=AF.Ln,
                                                 bias=eps_sb)
                            ent = work.tile([128, 1], F32, name=f"ent{k}",
                                            tag=f"ent{k}", bufs=2)
                            nc.vector.tensor_mul(out=ln_t, in0=p_k, in1=ln_t)
                            nc.vector.reduce_sum(out=ent, in_=ln_t,
                                                 axis=mybir.AxisListType.X)
                            attn.append(p_k)
                            ents.append(ent)

                        u0 = work.tile([128, 1], F32, name="u0", tag="u0", bufs=2)
                        u1 = work.tile([128, 1], F32, name="u1", tag="u1", bufs=2)
                        nc.vector.tensor_scalar_mul(out=u0, in0=ents[0],
                                                    scalar1=-temp)
                        nc.vector.tensor_scalar_mul(out=u1, in0=ents[1],
                                                    scalar1=-temp)
                        r0 = work.tile([128, 1], F32, name="r0", tag="r0", bufs=2)
                        r1 = work.tile([128, 1], F32, name="r1", tag="r1", bufs=2)
                        nc.vector.tensor_tensor(out=r0, in0=u0, in1=u1, op=OP.is_le)
                        nc.vector.tensor_tensor(out=r1, in0=u0, in1=u1, op=OP.is_gt)
                        nc.vector.tensor_tensor(
                            out=heat_st[:, b * NT + nt: b * NT + nt + 1],
                            in0=u0, in1=u1, op=OP.min)

                        nc.vector.tensor_scalar_mul(out=attn[0], in0=attn[0],
                                                    scalar1=r0)
                        nc.vector.tensor_scalar_mul(out=attn[1], in0=attn[1],
                                                    scalar1=r1)
                        acsel = work.tile([128, N], F32R, name="acsel",
                                          tag="acsel", bufs=1)
                        nc.vector.tensor_add(out=acsel, in0=attn[0], in1=attn[1])

                        pst = psum_big.tile([128, N], F32R, name="pst", tag="pbig")
                        for mb in range(NT):
                            nc.tensor.transpose(
                                pst[:, mb * 128:(mb + 1) * 128],
                                acsel[:, mb * 128:(mb + 1) * 128],
                                ident,
                            )
                        acT = work.tile([128, NT, 128], F32R, name="acT",
                                        tag="acT", bufs=1)
                        nc.scalar.copy(out=acT, in_=pst[:])

                        pso = psum_out.tile([128, D], F32, name="pso", tag="pso")
                        for mb in range(NT):
                            nc.tensor.matmul(
                                pso[:, 0:512], acT[:, mb], yn[:, mb, 0:512],
                                start=(mb == 0), stop=(mb == NT - 1),
                            )
                            nc.tensor.matmul(
                                pso[:, 512:D], acT[:, mb], yn[:, mb, 512:D],
                                start=(mb == 0), stop=(mb == NT - 1),
                            )
                        out_sb = work.tile([128, D], F32, name="out_sb",
                                           tag="out_sb", bufs=2)
                        nc.vector.tensor_copy(out_sb, pso[:])
                        nc.sync.dma_start(
                            out=out_d[b, nt * 128:(nt + 1) * 128, :], in_=out_sb)

                hsig = singles.tile([128, BPC * NT], F32, name="hsig")
                nc.scalar.activation(out=hsig, in_=heat_st, func=AF.Sigmoid)
                nc.vector.tensor_scalar(out=hsig, in0=hsig, scalar1=-2.0,
                                        scalar2=2.0, op0=OP.mult, op1=OP.add)
                for b in range(BPC):
                    nc.sync.dma_start(
                        out=heat_d[b].rearrange("(t p) -> p t", p=128),
                        in_=hsig[:, b * NT:(b + 1) * NT])

    _split_waits(nc)
    return nc


def _prep(inputs):
    x = np.asarray(inputs["x"], dtype=np.float32)
    y = np.asarray(inputs["y"], dtype=np.float32)
    coords = np.asarray(inputs["coords"], dtype=np.float32)
    U = np.asarray(inputs["U"], dtype=np.float64)
    S1 = np.asarray(inputs["S1"], dtype=np.float64)
    S2 = np.asarray(inputs["S2"], dtype=np.float64)
    gating = float(np.asarray(inputs["gating"]))
    temp = float(np.asarray(inputs["temp"]))
    pos_emb = np.asarray(inputs["pos_emb"], dtype=np.float32)

    g = float(1.0 / (1.0 + np.exp(-gating)))
    W1 = (U.T @ (S1[:, None] ** 2 * U)).astype(np.float32)
    W2 = (U.T @ (S2[:, None] ** 2 * U)).astype(np.float32)
    W = np.stack([W1, W2])

    pe = pos_emb[:, :, 0]                               # [N, 6]
    # diags[nt, p, c*128+j] = pe[nt*128+p, c] * (p == j)
    eye = np.eye(128, dtype=np.float32)
    pe_r = pe.reshape(NT, 128, 6)
    diags = (pe_r[:, :, :, None] * eye[None, :, None, :]).reshape(
        NT, 128, 6 * 128)
    diags = np.ascontiguousarray(diags)

    xT = np.ascontiguousarray(x.transpose(0, 2, 1))
    yT = np.ascontiguousarray(y.transpose(0, 2, 1))
    co = np.ascontiguousarray(coords.reshape(N, N * 6))

    in_maps = []
    for c in range(NCORES):
        sl = slice(c * BPC, (c + 1) * BPC)
        in_maps.append({
            "xT": xT[sl], "yT": yT[sl], "y": y[sl],
            "coords": co, "diags": diags, "ident": eye, "W": W,
        })
    return in_maps, g, temp


def run(inputs, trace=False, trace_kwargs=None):
    in_maps, g, temp = _prep(inputs)
    nc = build_nc(g, temp)
    br = run_bass_kernel_spmd(
        nc, in_maps, core_ids=list(range(NCORES)), trace=trace,
        **(trace_kwargs or {}),
    )
    out = np.concatenate([r["out"] for r in br.results], axis=0)
    heat = np.concatenate([r["heat"] for r in br.results], axis=0)
    heat = heat.reshape(B, N, 1).astype(np.float32)
    return (out, heat), br


def kernel(**inputs):
    (out, heat), _ = run(inputs, trace=False)
    return (out, heat)


# revision 10
# speedup vs baseline: 1.4580x; 1.4580x over previous
"""Trainium2 Bass kernel for nn_CrossAttention_Sp (moe_routing).

  W_k = U.T @ diag(S_k^2) @ U                       (host, tiny)
  x_k = x @ W_k ;  patch_k = softmax(x_k @ y.T * d^-0.5)
  q = softmax(coords @ pos_emb)  (shared over batch/k)
  attn_k = (1-g)*patch_k + g*q ;  e_k = -sum(attn_k*ln(attn_k+1e-8))
  route k* = argmin_k temp*e_k (ties k=0); heat = 2-2*sigmoid(min temp*e_k)
  out = attn_{k*} @ y

Data-parallel over B=16 across 8 cores. Matmuls in float32r (~4x faster
than fp32 on PE, ~1.6e-4 rel err). fp32r operand tiles are loaded via DMA
from fp32r DRAM params or written as fp32r by ACT/DVE (BIR verifier
requires producers of fp32r-matmul operands to emit fp32r).
"""

import numpy as np

import concourse.bass as bass
import concourse.mybir as mybir
import concourse.tile as tile
from concourse.bass_utils import run_bass_kernel_spmd

B, N, D = 16, 1024, 768
NCORES = 8
BPC = B // NCORES
NT = N // 128
DC = D // 128
F32 = mybir.dt.float32
F32R = mybir.dt.float32r
BF16 = mybir.dt.bfloat16
AF = mybir.ActivationFunctionType
OP = mybir.AluOpType


def _split_waits(nc, max_waits=1):
    """This toolchain's walrus accepts at most one sync-wait per instruction;
    split extras onto EventSemaphore nops on the same engine."""
    ctr = 0
    for func in nc.m.functions:
        for block in func.blocks:
            new_insts = []
            for inst in block.instructions:
                si = inst.sync_info
                if si is not None and len(si.on_wait) > max_waits:
                    waits = list(si.on_wait)
                    extra, keep = waits[:-max_waits], waits[-max_waits:]
                    while extra:
                        chunk, extra = extra[:max_waits], extra[max_waits:]
                        ctr += 1
                        new_insts.append(mybir.InstEventSemaphore(
                            name=f"I-waitsplit-{ctr}",
                            engine=inst.engine,
                            sync_info=mybir.SyncInfo(on_wait=chunk, on_update=[]),
                        ))
                    si.on_wait = keep
                new_insts.append(inst)
            block.instructions = new_insts
    return ctr


def build_nc(g: float, temp: float):
    one_m_g = 1.0 - g
    scale = float(D) ** -0.5

    nc = bass.Bass()
    xT_d = nc.declare_dram_parameter("xT", [BPC, D, N], F32R, isOutput=False)
    yT_d = nc.declare_dram_parameter("yT", [BPC, D, N], F32R, isOutput=False)
    y_d = nc.declare_dram_parameter("y", [BPC, N, D], F32R, isOutput=False)
    co_d = nc.declare_dram_parameter("coords", [N, N * 6], F32R, isOutput=False)
    dgs_d = nc.declare_dram_parameter("diags", [NT, 128, 6 * 128], F32R,
                                      isOutput=False)
    id_d = nc.declare_dram_parameter("ident", [128, 128], F32R, isOutput=False)
    w_d = nc.declare_dram_parameter("W", [2, D, D], F32R, isOutput=False)
    out_d = nc.declare_dram_parameter("out", [BPC, N, D], F32, isOutput=True)
    heat_d = nc.declare_dram_parameter("heat", [BPC, N], F32, isOutput=True)

    with tile.TileContext(nc) as tc:
        with (
            tc.tile_pool(name="singles", bufs=1) as singles,
            tc.tile_pool(name="gqp", bufs=1) as gqp,
            tc.tile_pool(name="psum_big", bufs=2, space="PSUM") as psum_big,
            tc.tile_pool(name="psum_out", bufs=1, space="PSUM") as psum_out,
        ):
            ident = singles.tile([128, 128], F32R, name="ident")
            nc.sync.dma_start(out=ident, in_=id_d[:])
            w_sb = singles.tile([128, 2, DC, D], F32R, name="w_sb")
            for k in range(2):
                nc.sync.dma_start(
                    out=w_sb[:, k],
                    in_=w_d[k].rearrange("(t p) d -> p t d", p=128),
                )
            heat_st = singles.tile([128, BPC * NT], F32, name="heat_st")
            eps_sb = singles.tile([128, 1], F32, name="eps_sb")
            nc.vector.memset(eps_sb, 1e-8)

            gq = [gqp.tile([128, N], BF16, name=f"gq{i}", tag=f"gq{i}")
                  for i in range(NT)]

            # ---- stage 1: pos scores (shared across batches and k) ----
            with (
                tc.tile_pool(name="s1", bufs=2) as s1,
                tc.tile_pool(name="s1s", bufs=2) as s1s,
            ):
                for nt in range(NT):
                    co_t = s1.tile([128, N * 6], F32R, name="co_t", tag="co")
                    nc.sync.dma_start(out=co_t,
                                      in_=co_d[nt * 128:(nt + 1) * 128, :])
                    co3 = co_t.rearrange("p (m c) -> p m c", c=6)
                    dg = s1s.tile([128, 6, 128], F32R, name="dg", tag="dg")
                    nc.sync.dma_start(out=dg, in_=dgs_d[nt].rearrange(
                        "p (c j) -> p c j", c=6))
                    ps = psum_big.tile([128, N], F32, name="ps_pos", tag="psz")
                    for h in range(2):
                        for c in range(6):
                            nc.tensor.matmul(
                                ps[:, h * 512:(h + 1) * 512],
                                dg[:, c],
                                co3[:, h * 512:(h + 1) * 512, c],
                                start=(c == 0), stop=(c == 5),
                            )
                    qx = s1s.tile([128, N], F32, name="qx", tag="qx")
                    qden = s1s.tile([128, 1], F32, name="qden", tag="qden")
                    nc.scalar.activation(out=qx, in_=ps, func=AF.Exp,
                                         accum_out=qden)
                    qinv = s1s.tile([128, 1], F32, name="qinv", tag="qinv")
                    nc.vector.reciprocal(out=qinv, in_=qden)
                    nc.vector.tensor_scalar(
                        out=gq[nt], in0=qx, scalar1=qinv, scalar2=g,
                        op0=OP.mult, op1=OP.mult,
                    )

            # ---- stage 2: per batch, software-pipelined over (b, nt) ----
            # logits+exp of item i+1 are issued before the blend/route/
            # transpose/out of item i, so the PE never waits on the
            # softmax/entropy chain (keeps HAM at full clock).
            with (
                tc.tile_pool(name="xy", bufs=2) as xy,
                tc.tile_pool(name="yp", bufs=1) as yp,
                tc.tile_pool(name="x12", bufs=1) as x12p,
                tc.tile_pool(name="work", bufs=1) as work,
            ):
                x12_b = {}
                yt_b = {}
                yn_b = {}

                def batch_setup(b):
                    xt = xy.tile([128, DC, N], F32R, name="xt", tag="xyT")
                    nc.sync.dma_start(
                        out=xt, in_=xT_d[b].rearrange("(t p) m -> p t m", p=128))
                    x12 = [[x12p.tile([128, N], F32R, name=f"x{k}T{do}",
                                      tag=f"x{k}T{do}")
                            for do in range(DC)] for k in range(2)]
                    for k in range(2):
                        for do in range(DC):
                            psx = psum_big.tile([128, N], F32, name="psx",
                                                tag="psz")
                            for di in range(DC):
                                for h in range(2):
                                    nc.tensor.matmul(
                                        psx[:, h * 512:(h + 1) * 512],
                                        w_sb[:, k, di, do * 128:(do + 1) * 128],
                                        xt[:, di, h * 512:(h + 1) * 512],
                                        start=(di == 0), stop=(di == DC - 1),
                                    )
                            nc.scalar.copy(out=x12[k][do], in_=psx[:])
                    yt = xy.tile([128, DC, N], F32R, name="yt", tag="xyT")
                    nc.sync.dma_start(
                        out=yt, in_=yT_d[b].rearrange("(t p) m -> p t m", p=128))
                    yn = yp.tile([128, NT, D], F32R, name="yn", tag="yn")
                    nc.sync.dma_start(
                        out=yn, in_=y_d[b].rearrange("(t p) d -> p t d", p=128))
                    x12_b[b], yt_b[b], yn_b[b] = x12, yt, yn

                def stage_logits(b, nt):
                    st = {}
                    for k in range(2):
                        psz = psum_big.tile([128, N], F32, name="psz", tag="psz")
                        for dc in range(DC):
                            for h in range(2):
                                nc.tensor.matmul(
                                    psz[:, h * 512:(h + 1) * 512],
                                    x12_b[b][k][dc][:, nt * 128:(nt + 1) * 128],
                                    yt_b[b][:, dc, h * 512:(h + 1) * 512],
                                    start=(dc == 0), stop=(dc == DC - 1),
                                )
                        p_k = work.tile([128, N], F32, name=f"p{k}",
                                        tag=f"p{k}", bufs=2)
                        den = work.tile([128, 1], F32, name=f"den{k}",
                                        tag=f"den{k}", bufs=2)
                        nc.scalar.activation(out=p_k, in_=psz, func=AF.Exp,
                                             scale=scale, accum_out=den)
                        inv = work.tile([128, 1], F32, name=f"inv{k}",
                                        tag=f"inv{k}", bufs=2)
                        nc.vector.reciprocal(out=inv, in_=den)
                        st[f"p{k}"] = p_k
                        st[f"inv{k}"] = inv
                    return st

                def stage_finish(b, nt, st):
                    attn = []
                    ents = []
                    for k in range(2):
                        p_k, inv = st[f"p{k}"], st[f"inv{k}"]
                        nc.vector.tensor_scalar(
                            out=p_k, in0=p_k, scalar1=inv, scalar2=one_m_g,
                            op0=OP.mult, op1=OP.mult,
                        )
                        nc.vector.tensor_add(out=p_k, in0=p_k, in1=gq[nt])
                        ln_t = work.tile([128, N], F32, name=f"ln{k}",
                                         tag="lnsel", bufs=2)
                        nc.scalar.activation(out=ln_t, in_=p_k, func=AF.Ln,
                                             bias=eps_sb)
                        ent = work.tile([128, 1], F32, name=f"ent{k}",
                                        tag=f"ent{k}", bufs=2)
                        nc.vector.tensor_mul(out=ln_t, in0=p_k, in1=ln_t)
                        nc.vector.reduce_sum(out=ent, in_=ln_t,
                                             axis=mybir.AxisListType.X)
                        attn.append(p_k)
                        ents.append(ent)

                    u0 = work.tile([128, 1], F32, name="u0", tag="u0", bufs=2)
                    u1 = work.tile([128, 1], F32, name="u1", tag="u1", bufs=2)
                    nc.vector.tensor_scalar_mul(out=u0, in0=ents[0], scalar1=-temp)
                    nc.vector.tensor_scalar_mul(out=u1, in0=ents[1], scalar1=-temp)
                    r0 = work.tile([128, 1], F32, name="r0", tag="r0", bufs=2)
                    r1 = work.tile([128, 1], F32, name="r1", tag="r1", bufs=2)
                    nc.vector.tensor_tensor(out=r0, in0=u0, in1=u1, op=OP.is_le)
                    nc.vector.tensor_tensor(out=r1, in0=u0, in1=u1, op=OP.is_gt)
                    nc.vector.tensor_tensor(
                        out=heat_st[:, b * NT + nt: b * NT + nt + 1],
                        in0=u0, in1=u1, op=OP.min)

                    nc.vector.tensor_scalar_mul(out=attn[0], in0=attn[0],
                                                scalar1=r0)
                    nc.vector.tensor_scalar_mul(out=attn[1], in0=attn[1],
                                                scalar1=r1)
                    acsel = work.tile([128, N], F32R, name="acsel",
                                      tag="lnsel", bufs=2)
                    nc.vector.tensor_add(out=acsel, in0=attn[0], in1=attn[1])

                    pst = psum_big.tile([128, N], F32R, name="pst", tag="pst",
                                        bufs=1)
                    for mb in range(NT):
                        nc.tensor.transpose(
                            pst[:, mb * 128:(mb + 1) * 128],
                            acsel[:, mb * 128:(mb + 1) * 128],
                            ident,
                        )
                    acT = work.tile([128, NT, 128], F32R, name="acT",
                                    tag="acT", bufs=1)
                    nc.scalar.copy(out=acT, in_=pst[:])

                    yn = yn_b[b]
                    pso = psum_out.tile([128, D], F32, name="pso", tag="pso",
                                        bufs=1)
                    for mb in range(NT):
                        nc.tensor.matmul(
                            pso[:, 0:512], acT[:, mb], yn[:, mb, 0:512],
                            start=(mb == 0), stop=(mb == NT - 1),
                        )
                        nc.tensor.matmul(
                            pso[:, 512:D], acT[:, mb], yn[:, mb, 512:D],
                            start=(mb == 0), stop=(mb == NT - 1),
                        )
                    out_sb = work.tile([128, D], F32, name="out_sb",
                                       tag="out_sb", bufs=2)
                    nc.vector.tensor_copy(out_sb, pso[:])
                    nc.sync.dma_start(
                        out=out_d[b, nt * 128:(nt + 1) * 128, :], in_=out_sb)

                pending = None
                for b in range(BPC):
                    batch_setup(b)
                    for nt in range(NT):
                        st = stage_logits(b, nt)
                        if pending is not None:
                            stage_finish(*pending)
                        pending = (b, nt, st)
                stage_finish(*pending)

                hsig = singles.tile([128, BPC * NT], F32, name="hsig")
                nc.scalar.activation(out=hsig, in_=heat_st, func=AF.Sigmoid)
                nc.vector.tensor_scalar(out=hsig, in0=hsig, scalar1=-2.0,
                                        scalar2=2.0, op0=OP.mult, op1=OP.add)
                for b in range(BPC):
                    nc.sync.dma_start(
                        out=heat_d[b].rearrange("(t p) -> p t", p=128),
                        in_=hsig[:, b * NT:(b + 1) * NT])

    _split_waits(nc)
    return nc


def _prep(inputs):
    x = np.asarray(inputs["x"], dtype=np.float32)
    y = np.asarray(inputs["y"], dtype=np.float32)
    coords = np.asarray(inputs["coords"], dtype=np.float32)
    U = np.asarray(inputs["U"], dtype=np.float64)
    S1 = np.asarray(inputs["S1"], dtype=np.float64)
    S2 = np.asarray(inputs["S2"], dtype=np.float64)
    gating = float(np.asarray(inputs["gating"]))
    temp = float(np.asarray(inputs["temp"]))
    pos_emb = np.asarray(inputs["pos_emb"], dtype=np.float32)

    g = float(1.0 / (1.0 + np.exp(-gating)))
    W1 = (U.T @ (S1[:, None] ** 2 * U)).astype(np.float32)
    W2 = (U.T @ (S2[:, None] ** 2 * U)).astype(np.float32)
    W = np.stack([W1, W2])

    pe = pos_emb[:, :, 0]                               # [N, 6]
    # diags[nt, p, c*128+j] = pe[nt*128+p, c] * (p == j)
    eye = np.eye(128, dtype=np.float32)
    pe_r = pe.reshape(NT, 128, 6)
    diags = (pe_r[:, :, :, None] * eye[None, :, None, :]).reshape(
        NT, 128, 6 * 128)
    diags = np.ascontiguousarray(diags)

    xT = np.ascontiguousarray(x.transpose(0, 2, 1))
    yT = np.ascontiguousarray(y.transpose(0, 2, 1))
    co = np.ascontiguousarray(coords.reshape(N, N * 6))

    in_maps = []
    for c in range(NCORES):
        sl = slice(c * BPC, (c + 1) * BPC)
        in_maps.append({
            "xT": xT[sl], "yT": yT[sl], "y": y[sl],
            "coords": co, "diags": diags, "ident": eye, "W": W,
        })
    return in_maps, g, temp


def run(inputs, trace=False, trace_kwargs=None):
    in_maps, g, temp = _prep(inputs)
    nc = build_nc(g, temp)
    br = run_bass_kernel_spmd(
        nc, in_maps, core_ids=list(range(NCORES)), trace=trace,
        **(trace_kwargs or {}),
    )
    out = np.concatenate([r["out"] for r in br.results], axis=0)
    heat = np.concatenate([r["heat"] for r in br.results], axis=0)
    heat = heat.reshape(B, N, 1).astype(np.float32)
    return (out, heat), br


def kernel(**inputs):
    (out, heat), _ = run(inputs, trace=False)
    return (out, heat)
